# revision 22
# baseline (speedup 1.0000x reference)
"""LongFormer dilated-window attention block on 8 trn2 NeuronCores.

Sharding: 8 cores = 4 dilation residues x 2 sequence halves. Query q
attends keys q + 4*j - 512 (j=0..256), i.e. only keys with the same
residue mod DIL=4. De-interleaving by residue turns the dilated window
into a contiguous +-128 sliding window in "residue space". Each core
gets a zero-padded [512, 512] slice of x: its 256 owned rows plus a
128-row halo on each side (all in residue space), so no communication
is needed.

v7 (vs the v2 baseline at ~72us):
- All inputs are host-permuted into their exact SBUF layouts; x is
  transposed on the host. wq/wk ship as j-major 128KB blocks and
  xeT/wv as halves, so the projection matmuls chase the DMA arrivals
  at fine granularity across both HWDGE queues (sync + scalar) with
  the emission order matched to the arrival order (engine queues are
  in-order FIFOs).
- A PE warm-up burst plus the gap-free projection pipeline un-throttles
  the HAM clock gate (1.2 -> 2.4 GHz) ~3.4us after the framework
  prologue and keeps it warm.
- PV is computed transposed (out[q, f] with the p tile as stationary),
  so the softmax denominator - obtained free via an 8.0 column appended
  to v - lands per-PARTITION. Normalization is one DVE reciprocal +
  per-partition-scale multiplies (split DVE/ACT); the eight fp32-HIGH
  rank-1 broadcast matmuls of the baseline and their reciprocal chains
  are gone.
- x1 returns to feature-major form for the FFN via eight PE transposes
  against a host-shipped identity; FFN1 accumulates inside the
  attention pipeline (chunk ec right after pv(j=ec)).
- Bias matmuls are gone: b1/b2 are zero in this problem (spec fill=
  zeros); a general variant (ACT per-partition bias for b1, host-
  replicated b2 tile + DVE add) is compiled only if a bias is nonzero.
- The residual add (x +) happens on the host in f32; the kernel output
  is bf16 (halves the tail DMA).
"""

import os
import sys

if "/opt/trn_rl_repo" not in sys.path:
    sys.path.insert(0, "/opt/trn_rl_repo")

import numpy as np

N_CORES = 8
S, E, H, FEAT = 2048, 512, 8, 64
DIL = 4
SC = 256      # owned queries per core (residue space)
EXT = 512     # ext rows per core (owned + 128 halo each side)
WARM_MM = int(os.environ.get("WARM_MM", "8"))

_CACHE = {}


def _build_nc(with_bias):
    import concourse.bacc as bacc
    import concourse.tile as tile
    import concourse.mybir as mybir
    import concourse.bass as bass

    dt = mybir.dt
    f32 = dt.float32
    bf16 = dt.bfloat16
    Alu = mybir.AluOpType
    Act = mybir.ActivationFunctionType

    nc = bacc.Bacc("TRN2", target_bir_lowering=False, debug=False,
                   num_devices=N_CORES)

    # ---- DRAM I/O (all host-prepared in final SBUF layout) ----
    xet_d = nc.dram_tensor("xet", [128, 4 * EXT], bf16,
                           kind="ExternalInput").ap()
    # wq/wk are j-major: [p, j*512 + o*128 + c] = W.T[o*128+p, j*128+c]
    wq_d = nc.dram_tensor("wq", [128, 4 * E], bf16, kind="ExternalInput").ap()
    wk_d = nc.dram_tensor("wk", [128, 4 * E], bf16, kind="ExternalInput").ap()
    wv_d = nc.dram_tensor("wv", [128, 4 * E], bf16, kind="ExternalInput").ap()
    w1_d = nc.dram_tensor("w1", [128, 4 * E], bf16, kind="ExternalInput").ap()
    w2_d = nc.dram_tensor("w2", [128, 4 * E], bf16, kind="ExternalInput").ap()
    idn_d = nc.dram_tensor("idn", [128, 128], bf16, kind="ExternalInput").ap()
    if with_bias:
        b1c_d = nc.dram_tensor("b1c", [128, 4], f32,
                               kind="ExternalInput").ap()
        b2r_d = nc.dram_tensor("b2r", [128, E], f32,
                               kind="ExternalInput").ap()
    out_d = nc.dram_tensor("out", [SC, E], bf16,
                           kind="ExternalOutput").ap()

    DBG = bool(os.environ.get("KDBG"))
    dbg = {}
    if DBG:
        for nm, shp, dt_ in [
            ("dqT", [128, 4, SC], bf16), ("dkT", [128, 4, EXT], bf16),
            ("dvsb", [128, 4, H, 65], bf16),
            ("dpA", [128, 3, 128], bf16), ("dpB", [128, 3, 128], bf16),
            ("dx1n", [128, 2, E], bf16), ("dx1T", [128, 4, SC], bf16),
            ("dfT", [128, 4, SC], bf16),
        ]:
            dbg[nm] = nc.dram_tensor(nm, shp, dt_,
                                     kind="ExternalOutput").ap()

    with tile.TileContext(nc) as tc:
        with (
            tc.tile_pool(name="singles", bufs=1) as singles,
            tc.tile_pool(name="ptiles", bufs=8) as ptiles,
            tc.tile_pool(name="rcps", bufs=4) as rcps,
            tc.tile_pool(name="ps_big", bufs=2, space="PSUM") as ps_big,
            tc.tile_pool(name="ps_sc", bufs=2, space="PSUM") as ps_sc,
            tc.tile_pool(name="ps_pv", bufs=2, space="PSUM") as ps_pv,
            tc.tile_pool(name="ps_ffn", bufs=2, space="PSUM") as ps_ffn,
        ):
            # ---- persistent SBUF tiles ----
            xeT = singles.tile([128, 4, EXT], bf16)     # [e_p, e_chunk, seq]
            # wq/wk j-major so each 128KB j-block DMA lands contiguously
            wq_sb = singles.tile([128, 4, 4, 128], bf16)  # [e_p, j, e_chunk, c]
            wk_sb = singles.tile([128, 4, 4, 128], bf16)
            wv_sb = singles.tile([128, 4, E], bf16)
            w1_sb = singles.tile([128, 4, E], bf16)
            w2_sb = singles.tile([128, 4, E], bf16)
            idn = singles.tile([128, 128], bf16)
            qT = singles.tile([128, 4, SC], bf16)       # [f_p, f_chunk, q]
            kT = singles.tile([128, 4, EXT], bf16)      # [f_p, f_chunk, key]
            # v natural per key chunk, per head, with an 8.0 column at 64
            # (folds the 1/sqrt(64) into the softmax denominator)
            v_sb = singles.tile([128, 4, H, 65], bf16)  # [key_p, ca, h, f|8]
            x1n = singles.tile([128, 2, E], bf16)       # [q_p, s, e]
            x1T = singles.tile([128, 4, SC], bf16)      # [e_p, e_chunk, q]
            fT = singles.tile([128, 4, SC], bf16)       # [f1_p, f1_chunk, q]
            gout = singles.tile([128, 2, E], bf16)      # relu(ffn2) out
            onesP = singles.tile([128, 128], bf16)      # warm-up stationary
            if with_bias:
                b1c_sb = singles.tile([128, 4], f32)
                b2r_sb = singles.tile([128, E], f32)

            # ---- input DMAs, finest-granularity-first so the
            # projection matmuls can chase the arrivals ----
            def qk_block(sb, dr, j):
                nc.sync.dma_start(
                    sb[:, :, 128 * j:128 * j + 128],
                    dr[:, 512 * j:512 * j + 512].rearrange(
                        "p (o c) -> p o c", o=4))

            def sc_half(sb, dr, h):
                nc.scalar.dma_start(
                    sb[:, 2 * h:2 * h + 2, :].rearrange("p a b -> p (a b)"),
                    dr[:, 1024 * h:1024 * h + 1024])

            qk_block(wq_sb, wq_d, 0)
            sc_half(xeT, xet_d, 0)
            qk_block(wk_sb, wk_d, 0)
            sc_half(xeT, xet_d, 1)
            qk_block(wq_sb, wq_d, 1)
            sc_half(wv_sb, wv_d, 0)
            qk_block(wk_sb, wk_d, 1)
            sc_half(wv_sb, wv_d, 1)
            qk_block(wq_sb, wq_d, 2)
            qk_block(wk_sb, wk_d, 2)
            nc.scalar.dma_start(idn[:], idn_d[:])
            qk_block(wq_sb, wq_d, 3)
            nc.scalar.dma_start(w2_sb[:].rearrange("p a b -> p (a b)"),
                                w2_d[:])
            qk_block(wk_sb, wk_d, 3)
            nc.sync.dma_start(w1_sb[:].rearrange("p a b -> p (a b)"),
                              w1_d[:])
            if with_bias:
                nc.sync.dma_start(b1c_sb[:], b1c_d[:])
                nc.sync.dma_start(b2r_sb[:], b2r_d[:])

            # ---- constants (gpsimd: keep DVE free) ----
            nc.gpsimd.memset(onesP[:], 1.0)
            nc.gpsimd.memset(v_sb[:, :, :, 64:65], 8.0)

            # ---- PE warm-up: un-throttle the HAM clock gate while the
            # first DMAs stream in ----
            if WARM_MM:
                wp = ps_big.tile([128, 512], f32, tag="big", name="wp")
                wide = bass.AP(tensor=onesP.tensor, offset=onesP.offset,
                               ap=[onesP[:].ap[0], [0, 4], [1, 128]])
                for i in range(WARM_MM):
                    nc.tensor.matmul(wp[:], onesP[:], wide,
                                     start=True, stop=(i == WARM_MM - 1))
                junk = singles.tile([1, 1], f32)
                nc.vector.tensor_copy(out=junk[:], in_=wp[0:1, 0:1])

            # ---- projections, ke-halved to chase the xeT halves ----
            qps = {}
            kps = {}

            def emit_qproj(j, keh, pool):
                if keh == 0:
                    qps[j] = pool.tile([128, 512], f32,
                                       tag=pool.name[3:], name="qp")
                for ke in (2 * keh, 2 * keh + 1):
                    nc.tensor.matmul(
                        qps[j][:, 0:SC], wq_sb[:, ke, 128 * j:128 * j + 128],
                        xeT[:, ke, 128:384],
                        start=(ke == 0), stop=(ke == 3))
                if keh == 1:
                    nc.vector.tensor_copy(out=qT[:, j, :],
                                          in_=qps.pop(j)[:, 0:SC])

            def emit_kproj(j, keh, pool):
                if keh == 0:
                    kps[j] = pool.tile([128, 512], f32,
                                       tag=pool.name[3:], name="kp")
                for ke in (2 * keh, 2 * keh + 1):
                    nc.tensor.matmul(
                        kps[j][:], wk_sb[:, ke, 128 * j:128 * j + 128],
                        xeT[:, ke, :],
                        start=(ke == 0), stop=(ke == 3))
                if keh == 1:
                    nc.vector.tensor_copy(out=kT[:, j, :], in_=kps.pop(j)[:])

            vps = {}

            def emit_vproj(pair, keh):
                # two key chunks (ca) per call, ke-halved for the wv halves
                for ca in (2 * pair, 2 * pair + 1):
                    if keh == 0:
                        vps[ca] = ps_ffn.tile([128, 512], f32, tag="f",
                                              name="vp")
                    for ke in (2 * keh, 2 * keh + 1):
                        nc.tensor.matmul(
                            vps[ca][:], xeT[:, ke, 128 * ca:128 * ca + 128],
                            wv_sb[:, ke, :],
                            start=(ke == 0), stop=(ke == 3))
                    if keh == 1:
                        vp = vps.pop(ca)
                        vh = vp[:].rearrange("p (h f) -> p h f", h=H)
                        nc.vector.tensor_copy(
                            out=v_sb[:, ca, 0:4, 0:64], in_=vh[:, 0:4, :])
                        nc.scalar.copy(
                            out=v_sb[:, ca, 4:8, 0:64], in_=vh[:, 4:8, :])

            def emit_scores(j):
                # per head: two one-bank score tiles, band-sparse:
                #   tileA = [ca0|s0, ca1|s1, ca1|s0]  (tri, tri, full)
                #   tileB = [ca3|s1, ca2|s0, ca2|s1]  (tri, tri, full)
                # the two triangles of a tile share one affine predicate.
                p_js = []
                for hh in range(2):
                    o = 64 * hh
                    k_ = lambda ca: kT[o:o + 64, j, 128 * ca:128 * ca + 128]
                    q_ = lambda s0, s1: qT[o:o + 64, j, 128 * s0:128 * s1]
                    spA = ps_sc.tile([128, 512], f32, tag="sc", name="spA")
                    nc.tensor.matmul(spA[:, 0:128], k_(0), q_(0, 1),
                                     start=True, stop=False)
                    qf = q_(0, 2)
                    qrev = bass.AP(tensor=qf.tensor, offset=qf.offset + 128,
                                   ap=[qf.ap[0], [-128, 2], [1, 128]])
                    nc.tensor.matmul(spA[:, 128:384], k_(1), qrev,
                                     start=False, stop=True)
                    pa = ptiles.tile([128, 3, 128], bf16, tag="p", name="pa")
                    nc.scalar.activation(
                        out=pa[:].rearrange("p a b -> p (a b)"),
                        in_=spA[:, 0:384], func=Act.Exp)
                    nc.gpsimd.affine_select(
                        out=pa[:, 0:2, :], in_=pa[:, 0:2, :],
                        compare_op=Alu.is_ge, fill=0.0,
                        base=0, channel_multiplier=1,
                        pattern=[[0, 2], [-1, 128]])
                    spB = ps_sc.tile([128, 512], f32, tag="sc", name="spB")
                    nc.tensor.matmul(spB[:, 0:128], k_(3), q_(1, 2),
                                     start=True, stop=False)
                    nc.tensor.matmul(spB[:, 128:384], k_(2), q_(0, 2),
                                     start=False, stop=True)
                    pb = ptiles.tile([128, 3, 128], bf16, tag="p", name="pb")
                    nc.scalar.activation(
                        out=pb[:].rearrange("p a b -> p (a b)"),
                        in_=spB[:, 0:384], func=Act.Exp)
                    nc.gpsimd.affine_select(
                        out=pb[:, 0:2, :], in_=pb[:, 0:2, :],
                        compare_op=Alu.is_ge, fill=0.0,
                        base=0, channel_multiplier=-1,
                        pattern=[[0, 2], [1, 128]])
                    if DBG and j == 0 and hh == 0:
                        nc.sync.dma_start(dbg["dpA"][:], pa[:])
                        nc.sync.dma_start(dbg["dpB"][:], pb[:])
                    p_js.append((pa, pb))
                return p_js

            def emit_pv(j, p_js):
                # transposed p@v: out[q, f] with the p tile stationary.
                # s0 accumulates into cols 0:65, s1 into 128:193; one
                # accumulation group per (j,hh) so the single leading
                # start=True clears the whole bank.
                for hh in range(2):
                    pa, pb = p_js[hh]
                    h = 2 * j + hh
                    pvt = ps_pv.tile([128, 512], f32, tag="pv", name="pvt")
                    mms = [
                        (pvt[:, 0:65], pa[:, 0, :], 0),
                        (pvt[:, 0:65], pa[:, 2, :], 1),
                        (pvt[:, 0:65], pb[:, 1, :], 2),
                        (pvt[:, 128:193], pa[:, 1, :], 1),
                        (pvt[:, 128:193], pb[:, 2, :], 2),
                        (pvt[:, 128:193], pb[:, 0, :], 3),
                    ]
                    for i, (o_, p_, ca) in enumerate(mms):
                        nc.tensor.matmul(o_, p_, v_sb[:, ca, h, :],
                                         start=(i == 0),
                                         stop=(i == len(mms) - 1))
                    # normalize: denominators (8*D) sit per-partition at
                    # cols 64 / 192; x1 = pv * (1/(8D)) folds the 1/sqrt(64)
                    rcp = rcps.tile([128, 2], f32, tag="rcp", name="rcp")
                    nc.vector.reciprocal(out=rcp[:], in_=pvt[:, 64:320:128])
                    for s in range(2):
                        dst = x1n[:, s, 128 * j + 64 * hh:
                                  128 * j + 64 * hh + 64]
                        if hh == 0:
                            nc.vector.tensor_scalar_mul(
                                out=dst, in0=pvt[:, 128 * s:128 * s + 64],
                                scalar1=rcp[:, s:s + 1])
                        else:
                            nc.scalar.activation(
                                out=dst, in_=pvt[:, 128 * s:128 * s + 64],
                                func=Act.Copy, scale=rcp[:, s:s + 1])

            def emit_x1t(j):
                # x1 chunk back to feature-major via PE transpose
                for s in range(2):
                    tp = ps_big.tile([128, 128], bf16, tag="big", name="tp")
                    nc.tensor.transpose(
                        tp[:], x1n[:, s, 128 * j:128 * j + 128],
                        idn[:])
                    if s == 0:
                        nc.vector.tensor_copy(
                            out=x1T[:, j, 128 * s:128 * s + 128], in_=tp[:])
                    else:
                        nc.scalar.copy(
                            out=x1T[:, j, 128 * s:128 * s + 128], in_=tp[:])

            # ---- FFN1: fT = relu(W1 @ x1T (+ b1)), accumulated chunk by
            # chunk as x1T chunks complete. Two u-halves per PSUM bank,
            # one accumulation group per bank. ----
            f_ps = {}

            def emit_ffn1_mms(ec):
                for u in range(4):
                    bk, hf = divmod(u, 2)
                    if ec == 0 and hf == 0:
                        f_ps[bk] = ps_ffn.tile([128, 512], f32, tag="f",
                                               name="fp")
                    nc.tensor.matmul(
                        f_ps[bk][:, 256 * hf:256 * hf + 256],
                        w1_sb[:, ec, 128 * u:128 * u + 128], x1T[:, ec, :],
                        start=(ec == 0 and hf == 0),
                        stop=(ec == 3 and hf == 1))

            def emit_ffn1_fin():
                # split across DVE and ACT so the tail chain shortens
                for u in range(4):
                    bk, hf = divmod(u, 2)
                    src = f_ps[bk][:, 256 * hf:256 * hf + 256]
                    if with_bias:
                        nc.scalar.activation(out=fT[:, u, :], in_=src,
                                             func=Act.Relu,
                                             bias=b1c_sb[:, u:u + 1])
                    elif u % 2 == 0:
                        nc.vector.tensor_scalar_max(out=fT[:, u, :],
                                                    in0=src, scalar1=0.0)
                    else:
                        nc.scalar.activation(out=fT[:, u, :], in_=src,
                                             func=Act.Relu)

            # ---- pipeline, emission order == dependency-arrival order
            # (PE queue is an in-order FIFO: an instruction emitted
            # before its input lands stalls everything behind it) ----
            emit_qproj(0, 0, ps_big)
            emit_kproj(0, 0, ps_big)
            emit_qproj(1, 0, ps_pv)
            emit_kproj(1, 0, ps_pv)
            emit_qproj(0, 1, ps_big)
            emit_kproj(0, 1, ps_big)
            emit_qproj(1, 1, ps_pv)
            emit_kproj(1, 1, ps_pv)
            emit_vproj(0, 0)
            scores = [emit_scores(0)]
            emit_qproj(2, 0, ps_big)
            emit_qproj(2, 1, ps_big)
            emit_kproj(2, 0, ps_big)
            emit_kproj(2, 1, ps_big)
            scores.append(emit_scores(1))
            emit_vproj(0, 1)
            emit_vproj(1, 0)
            emit_vproj(1, 1)
            emit_pv(0, scores[0])
            emit_qproj(3, 0, ps_big)
            emit_qproj(3, 1, ps_big)
            emit_kproj(3, 0, ps_big)
            emit_kproj(3, 1, ps_big)
            emit_x1t(0)
            scores.append(emit_scores(2))
            emit_pv(1, scores[1])
            emit_x1t(1)
            scores.append(emit_scores(3))
            emit_ffn1_mms(0)
            emit_ffn1_mms(1)
            emit_pv(2, scores[2])
            emit_x1t(2)
            emit_ffn1_mms(2)
            emit_pv(3, scores[3])
            emit_x1t(3)
            emit_ffn1_mms(3)
            emit_ffn1_fin()

            # ---- FFN2 + relu (residual added on host) ----
            for qc in range(2):
                gp = ps_big.tile([128, 512], f32, tag="big", name="gp")
                for u in range(4):
                    nc.tensor.matmul(
                        gp[:], fT[:, u, 128 * qc:128 * qc + 128],
                        w2_sb[:, u, :],
                        start=(u == 0), stop=(u == 3))
                if with_bias:
                    nc.vector.tensor_tensor(out=gp[:], in0=gp[:],
                                            in1=b2r_sb[:], op=Alu.add)
                if qc == 0:
                    nc.vector.tensor_scalar_max(out=gout[:, qc, :],
                                                in0=gp[:], scalar1=0.0)
                    nc.sync.dma_start(
                        out_d[128 * qc:128 * qc + 128, :], gout[:, qc, :])
                else:
                    nc.scalar.activation(out=gout[:, qc, :], in_=gp[:],
                                         func=Act.Relu)
                    nc.scalar.dma_start(
                        out_d[128 * qc:128 * qc + 128, :], gout[:, qc, :])
            if DBG:
                nc.sync.dma_start(dbg["dqT"][:], qT[:])
                nc.sync.dma_start(dbg["dkT"][:], kT[:])
                nc.sync.dma_start(dbg["dvsb"][:], v_sb[:])
                nc.sync.dma_start(dbg["dx1n"][:], x1n[:])
                nc.sync.dma_start(dbg["dx1T"][:], x1T[:])
                nc.sync.dma_start(dbg["dfT"][:], fT[:])

    nc.compile()
    return nc


def _get_nc(with_bias):
    key = ("nc", with_bias)
    if key not in _CACHE:
        _CACHE[key] = _build_nc(with_bias)
    return _CACHE[key]


def _to_chunked(w):
    # [512, 512] -> [128, 4*512] with row p = [W[0*128+p,:], W[1*128+p,:],..]
    import ml_dtypes
    bf = ml_dtypes.bfloat16
    return np.ascontiguousarray(
        w.reshape(4, 128, 512).transpose(1, 0, 2).reshape(128, 2048)
        .astype(bf))


def _to_jblocks(w):
    # [512, 512] -> [128, 4j * (4o * 128c)]: j-major blocks, o-major inside
    import ml_dtypes
    bf = ml_dtypes.bfloat16
    # w[o*128+p, j*128+c] -> out[p, j, o, c]
    t = w.reshape(4, 128, 4, 128).transpose(1, 2, 0, 3)
    return np.ascontiguousarray(t.reshape(128, 2048).astype(bf))


def _shard_inputs(x, Wq, Wk, Wv, W1, b1, W2, b2, with_bias):
    import ml_dtypes
    bf = ml_dtypes.bfloat16
    x2 = np.ascontiguousarray(np.asarray(x, dtype=np.float32).reshape(S, E))
    ws = {
        "wq": _to_jblocks(np.asarray(Wq, np.float32).T),
        "wk": _to_jblocks(np.asarray(Wk, np.float32).T),
        "wv": _to_chunked(np.asarray(Wv, np.float32).T),
        "w1": _to_chunked(np.asarray(W1, np.float32).T),
        "w2": _to_chunked(np.asarray(W2, np.float32).T),
        "idn": np.ascontiguousarray(np.eye(128, dtype=np.float32).astype(bf)),
    }
    if with_bias:
        ws["b1c"] = np.ascontiguousarray(
            np.asarray(b1, np.float32).reshape(4, 128).T)
        ws["b2r"] = np.ascontiguousarray(np.broadcast_to(
            np.asarray(b2, np.float32).reshape(1, E), (128, E)))
    in_maps = []
    for c in range(N_CORES):
        r_, half = c >> 1, c & 1
        eidx = np.arange(256 * half - 128, 256 * half + 384)
        valid = (eidx >= 0) & (eidx < S // DIL)
        xe = np.zeros((EXT, E), np.float32)
        xe[valid] = x2[DIL * eidx[valid] + r_]
        in_maps.append({"xet": _to_chunked(xe.T), **ws})
    return in_maps


def _gather_outputs(results, x):
    x2 = np.asarray(x, np.float32).reshape(S, E)
    out = np.zeros((S, E), np.float32)
    for c in range(N_CORES):
        r_, half = c >> 1, c & 1
        i = np.arange(256 * half, 256 * half + SC)
        rows = DIL * i + r_
        out[rows] = np.asarray(results[c]["out"], np.float32) + x2[rows]
    return out.reshape(1, S, E)


def run(inputs, trace=False, tmpdir=None):
    from concourse import bass_utils
    with_bias = bool(np.any(inputs["b1"])) or bool(np.any(inputs["b2"]))
    nc = _get_nc(with_bias)
    in_maps = _shard_inputs(**inputs, with_bias=with_bias)
    res = bass_utils.run_bass_kernel_spmd(
        nc, in_maps, list(range(N_CORES)), trace=trace, tmpdir=tmpdir)
    return _gather_outputs(res.results, inputs["x"]), res


def kernel(x, Wq, Wk, Wv, W1, b1, W2, b2):
    out, _ = run(dict(x=x, Wq=Wq, Wk=Wk, Wv=Wv, W1=W1, b1=b1, W2=W2, b2=b2))
    return out


# revision 25
# speedup vs baseline: 1.0776x; 1.0776x over previous
"""LongFormer dilated-window attention block on 8 trn2 NeuronCores.

Sharding: 8 cores = 4 dilation residues x 2 sequence halves. Query q
attends keys q + 4*j - 512 (j=0..256), i.e. only keys with the same
residue mod DIL=4. De-interleaving by residue turns the dilated window
into a contiguous +-128 sliding window in "residue space". Each core
gets a zero-padded [512, 512] slice of x: its 256 owned rows plus a
128-row halo on each side (all in residue space), so no communication
is needed.

v7 (vs the v2 baseline at ~72us):
- All inputs are host-permuted into their exact SBUF layouts; x is
  transposed on the host. wq/wk ship as j-major 128KB blocks and
  xeT/wv as halves, so the projection matmuls chase the DMA arrivals
  at fine granularity across both HWDGE queues (sync + scalar) with
  the emission order matched to the arrival order (engine queues are
  in-order FIFOs).
- A PE warm-up burst plus the gap-free projection pipeline un-throttles
  the HAM clock gate (1.2 -> 2.4 GHz) ~3.4us after the framework
  prologue and keeps it warm.
- PV is computed transposed (out[q, f] with the p tile as stationary),
  so the softmax denominator - obtained free via an 8.0 column appended
  to v - lands per-PARTITION. Normalization is one DVE reciprocal +
  per-partition-scale multiplies (split DVE/ACT); the eight fp32-HIGH
  rank-1 broadcast matmuls of the baseline and their reciprocal chains
  are gone.
- x1 returns to feature-major form for the FFN via eight PE transposes
  against a host-shipped identity; FFN1 accumulates inside the
  attention pipeline (chunk ec right after pv(j=ec)).
- Bias matmuls are gone: b1/b2 are zero in this problem (spec fill=
  zeros); a general variant (ACT per-partition bias for b1, host-
  replicated b2 tile + DVE add) is compiled only if a bias is nonzero.
- The residual add (x +) happens on the host in f32; the kernel output
  is bf16 (halves the tail DMA).
"""

import os
import sys

if "/opt/trn_rl_repo" not in sys.path:
    sys.path.insert(0, "/opt/trn_rl_repo")

import numpy as np

N_CORES = 8
S, E, H, FEAT = 2048, 512, 8, 64
DIL = 4
SC = 256      # owned queries per core (residue space)
EXT = 512     # ext rows per core (owned + 128 halo each side)
WARM_MM = int(os.environ.get("WARM_MM", "8"))

_CACHE = {}


def _build_nc(with_bias):
    import concourse.bacc as bacc
    import concourse.tile as tile
    import concourse.mybir as mybir
    import concourse.bass as bass

    dt = mybir.dt
    f32 = dt.float32
    bf16 = dt.bfloat16
    Alu = mybir.AluOpType
    Act = mybir.ActivationFunctionType

    nc = bacc.Bacc("TRN2", target_bir_lowering=False, debug=False,
                   num_devices=N_CORES)

    # ---- DRAM I/O (all host-prepared in final SBUF layout) ----
    xet_d = nc.dram_tensor("xet", [128, 4 * EXT], bf16,
                           kind="ExternalInput").ap()
    # wq/wk are j-major: [p, j*512 + o*128 + c] = W.T[o*128+p, j*128+c]
    wq_d = nc.dram_tensor("wq", [128, 4 * E], bf16, kind="ExternalInput").ap()
    wk_d = nc.dram_tensor("wk", [128, 4 * E], bf16, kind="ExternalInput").ap()
    wv_d = nc.dram_tensor("wv", [128, 4 * E], bf16, kind="ExternalInput").ap()
    w1_d = nc.dram_tensor("w1", [128, 4 * E], bf16, kind="ExternalInput").ap()
    w2_d = nc.dram_tensor("w2", [128, 4 * E], bf16, kind="ExternalInput").ap()
    idn_d = nc.dram_tensor("idn", [128, 128], bf16, kind="ExternalInput").ap()
    if with_bias:
        b1c_d = nc.dram_tensor("b1c", [128, 4], f32,
                               kind="ExternalInput").ap()
        b2r_d = nc.dram_tensor("b2r", [128, E], f32,
                               kind="ExternalInput").ap()
    out_d = nc.dram_tensor("out", [SC, E], bf16,
                           kind="ExternalOutput").ap()

    DBG = bool(os.environ.get("KDBG"))
    dbg = {}
    if DBG:
        for nm, shp, dt_ in [
            ("dqT", [128, 4, SC], bf16), ("dkT", [128, 4, EXT], bf16),
            ("dvsb", [128, 4, H, 65], bf16),
            ("dpA", [128, 3, 128], bf16), ("dpB", [128, 3, 128], bf16),
            ("dx1n", [128, 2, E], bf16), ("dx1T", [128, 4, SC], bf16),
            ("dfT", [128, 4, SC], bf16),
        ]:
            dbg[nm] = nc.dram_tensor(nm, shp, dt_,
                                     kind="ExternalOutput").ap()

    with tile.TileContext(nc) as tc:
        with (
            tc.tile_pool(name="singles", bufs=1) as singles,
            tc.tile_pool(name="ptiles", bufs=8) as ptiles,
            tc.tile_pool(name="rcps", bufs=4) as rcps,
            tc.tile_pool(name="ps_big", bufs=2, space="PSUM") as ps_big,
            tc.tile_pool(name="ps_sc", bufs=2, space="PSUM") as ps_sc,
            tc.tile_pool(name="ps_pv", bufs=2, space="PSUM") as ps_pv,
            tc.tile_pool(name="ps_ffn", bufs=2, space="PSUM") as ps_ffn,
        ):
            # ---- persistent SBUF tiles ----
            xeT = singles.tile([128, 4, EXT], bf16)     # [e_p, e_chunk, seq]
            # wq/wk j-major so each 128KB j-block DMA lands contiguously
            wq_sb = singles.tile([128, 4, 4, 128], bf16)  # [e_p, j, e_chunk, c]
            wk_sb = singles.tile([128, 4, 4, 128], bf16)
            wv_sb = singles.tile([128, 4, E], bf16)
            w1_sb = singles.tile([128, 4, E], bf16)
            w2_sb = singles.tile([128, 4, E], bf16)
            idn = singles.tile([128, 128], bf16)
            qT = singles.tile([128, 4, SC], bf16)       # [f_p, f_chunk, q]
            kT = singles.tile([128, 4, EXT], bf16)      # [f_p, f_chunk, key]
            # v natural per key chunk, per head, with an 8.0 column at 64
            # (folds the 1/sqrt(64) into the softmax denominator)
            v_sb = singles.tile([128, 4, H, 65], bf16)  # [key_p, ca, h, f|8]
            x1n = singles.tile([128, 2, E], bf16)       # [q_p, s, e]
            x1T = singles.tile([128, 4, SC], bf16)      # [e_p, e_chunk, q]
            fT = singles.tile([128, 4, SC], bf16)       # [f1_p, f1_chunk, q]
            gout = singles.tile([128, 2, E], bf16)      # relu(ffn2) out
            onesP = singles.tile([128, 128], bf16)      # warm-up stationary
            if with_bias:
                b1c_sb = singles.tile([128, 4], f32)
                b2r_sb = singles.tile([128, E], f32)

            # ---- input DMAs, finest-granularity-first so the
            # projection matmuls can chase the arrivals ----
            def qk_block(sb, dr, j):
                nc.sync.dma_start(
                    sb[:, j, :, :].rearrange("p a b -> p (a b)"),
                    dr[:, 512 * j:512 * j + 512])

            def sc_half(sb, dr, h):
                nc.scalar.dma_start(
                    sb[:, 2 * h:2 * h + 2, :].rearrange("p a b -> p (a b)"),
                    dr[:, 1024 * h:1024 * h + 1024])

            qk_block(wq_sb, wq_d, 0)
            sc_half(xeT, xet_d, 0)
            qk_block(wk_sb, wk_d, 0)
            sc_half(xeT, xet_d, 1)
            qk_block(wq_sb, wq_d, 1)
            sc_half(wv_sb, wv_d, 0)
            qk_block(wk_sb, wk_d, 1)
            sc_half(wv_sb, wv_d, 1)
            qk_block(wq_sb, wq_d, 2)
            qk_block(wk_sb, wk_d, 2)
            nc.scalar.dma_start(idn[:], idn_d[:])
            qk_block(wq_sb, wq_d, 3)
            nc.scalar.dma_start(w2_sb[:].rearrange("p a b -> p (a b)"),
                                w2_d[:])
            qk_block(wk_sb, wk_d, 3)
            nc.sync.dma_start(w1_sb[:].rearrange("p a b -> p (a b)"),
                              w1_d[:])
            if with_bias:
                nc.sync.dma_start(b1c_sb[:], b1c_d[:])
                nc.sync.dma_start(b2r_sb[:], b2r_d[:])

            # ---- constants (gpsimd: keep DVE free) ----
            nc.gpsimd.memset(onesP[:], 1.0)
            nc.gpsimd.memset(v_sb[:, :, :, 64:65], 8.0)

            # ---- PE warm-up: un-throttle the HAM clock gate while the
            # first DMAs stream in ----
            if WARM_MM:
                wp = ps_big.tile([128, 512], f32, tag="big", name="wp")
                wide = bass.AP(tensor=onesP.tensor, offset=onesP.offset,
                               ap=[onesP[:].ap[0], [0, 4], [1, 128]])
                for i in range(WARM_MM):
                    nc.tensor.matmul(wp[:], onesP[:], wide,
                                     start=True, stop=(i == WARM_MM - 1))
                junk = singles.tile([1, 1], f32)
                nc.vector.tensor_copy(out=junk[:], in_=wp[0:1, 0:1])

            # ---- projections, ke-halved to chase the xeT halves ----
            qps = {}
            kps = {}

            def emit_qproj(j, keh, pool):
                if keh == 0:
                    qps[j] = pool.tile([128, 512], f32,
                                       tag=pool.name[3:], name="qp")
                for ke in (2 * keh, 2 * keh + 1):
                    nc.tensor.matmul(
                        qps[j][:, 0:SC], wq_sb[:, j, ke, :],
                        xeT[:, ke, 128:384],
                        start=(ke == 0), stop=(ke == 3))
                if keh == 1:
                    nc.vector.tensor_copy(out=qT[:, j, :],
                                          in_=qps.pop(j)[:, 0:SC])

            def emit_kproj(j, keh, pool):
                if keh == 0:
                    kps[j] = pool.tile([128, 512], f32,
                                       tag=pool.name[3:], name="kp")
                for ke in (2 * keh, 2 * keh + 1):
                    nc.tensor.matmul(
                        kps[j][:], wk_sb[:, j, ke, :],
                        xeT[:, ke, :],
                        start=(ke == 0), stop=(ke == 3))
                if keh == 1:
                    nc.vector.tensor_copy(out=kT[:, j, :], in_=kps.pop(j)[:])

            vps = {}

            def emit_vproj(pair, keh):
                # two key chunks (ca) per call, ke-halved for the wv halves
                for ca in (2 * pair, 2 * pair + 1):
                    if keh == 0:
                        vps[ca] = ps_ffn.tile([128, 512], f32, tag="f",
                                              name="vp")
                    for ke in (2 * keh, 2 * keh + 1):
                        nc.tensor.matmul(
                            vps[ca][:], xeT[:, ke, 128 * ca:128 * ca + 128],
                            wv_sb[:, ke, :],
                            start=(ke == 0), stop=(ke == 3))
                    if keh == 1:
                        vp = vps.pop(ca)
                        vh = vp[:].rearrange("p (h f) -> p h f", h=H)
                        nc.vector.tensor_copy(
                            out=v_sb[:, ca, 0:4, 0:64], in_=vh[:, 0:4, :])
                        nc.scalar.copy(
                            out=v_sb[:, ca, 4:8, 0:64], in_=vh[:, 4:8, :])

            def emit_scores(j):
                # per head: two one-bank score tiles, band-sparse:
                #   tileA = [ca0|s0, ca1|s1, ca1|s0]  (tri, tri, full)
                #   tileB = [ca3|s1, ca2|s0, ca2|s1]  (tri, tri, full)
                # the two triangles of a tile share one affine predicate.
                p_js = []
                for hh in range(2):
                    o = 64 * hh
                    k_ = lambda ca: kT[o:o + 64, j, 128 * ca:128 * ca + 128]
                    q_ = lambda s0, s1: qT[o:o + 64, j, 128 * s0:128 * s1]
                    spA = ps_sc.tile([128, 512], f32, tag="sc", name="spA")
                    nc.tensor.matmul(spA[:, 0:128], k_(0), q_(0, 1),
                                     start=True, stop=False)
                    qf = q_(0, 2)
                    qrev = bass.AP(tensor=qf.tensor, offset=qf.offset + 128,
                                   ap=[qf.ap[0], [-128, 2], [1, 128]])
                    nc.tensor.matmul(spA[:, 128:384], k_(1), qrev,
                                     start=False, stop=True)
                    pa = ptiles.tile([128, 3, 128], bf16, tag="p", name="pa")
                    nc.scalar.activation(
                        out=pa[:].rearrange("p a b -> p (a b)"),
                        in_=spA[:, 0:384], func=Act.Exp)
                    nc.gpsimd.affine_select(
                        out=pa[:, 0:2, :], in_=pa[:, 0:2, :],
                        compare_op=Alu.is_ge, fill=0.0,
                        base=0, channel_multiplier=1,
                        pattern=[[0, 2], [-1, 128]])
                    spB = ps_sc.tile([128, 512], f32, tag="sc", name="spB")
                    nc.tensor.matmul(spB[:, 0:128], k_(3), q_(1, 2),
                                     start=True, stop=False)
                    nc.tensor.matmul(spB[:, 128:384], k_(2), q_(0, 2),
                                     start=False, stop=True)
                    pb = ptiles.tile([128, 3, 128], bf16, tag="p", name="pb")
                    nc.scalar.activation(
                        out=pb[:].rearrange("p a b -> p (a b)"),
                        in_=spB[:, 0:384], func=Act.Exp)
                    nc.gpsimd.affine_select(
                        out=pb[:, 0:2, :], in_=pb[:, 0:2, :],
                        compare_op=Alu.is_ge, fill=0.0,
                        base=0, channel_multiplier=-1,
                        pattern=[[0, 2], [1, 128]])
                    if DBG and j == 0 and hh == 0:
                        nc.sync.dma_start(dbg["dpA"][:], pa[:])
                        nc.sync.dma_start(dbg["dpB"][:], pb[:])
                    p_js.append((pa, pb))
                return p_js

            def emit_pv(j, p_js):
                # transposed p@v: out[q, f] with the p tile stationary.
                # s0 accumulates into cols 0:65, s1 into 128:193; one
                # accumulation group per (j,hh) so the single leading
                # start=True clears the whole bank.
                for hh in range(2):
                    pa, pb = p_js[hh]
                    h = 2 * j + hh
                    pvt = ps_pv.tile([128, 512], f32, tag="pv", name="pvt")
                    mms = [
                        (pvt[:, 0:65], pa[:, 0, :], 0),
                        (pvt[:, 0:65], pa[:, 2, :], 1),
                        (pvt[:, 0:65], pb[:, 1, :], 2),
                        (pvt[:, 128:193], pa[:, 1, :], 1),
                        (pvt[:, 128:193], pb[:, 2, :], 2),
                        (pvt[:, 128:193], pb[:, 0, :], 3),
                    ]
                    for i, (o_, p_, ca) in enumerate(mms):
                        nc.tensor.matmul(o_, p_, v_sb[:, ca, h, :],
                                         start=(i == 0),
                                         stop=(i == len(mms) - 1))
                    # normalize: denominators (8*D) sit per-partition at
                    # cols 64 / 192; x1 = pv * (1/(8D)) folds the 1/sqrt(64)
                    rcp = rcps.tile([128, 2], f32, tag="rcp", name="rcp")
                    nc.vector.reciprocal(out=rcp[:], in_=pvt[:, 64:320:128])
                    for s in range(2):
                        dst = x1n[:, s, 128 * j + 64 * hh:
                                  128 * j + 64 * hh + 64]
                        if hh == 0:
                            nc.vector.tensor_scalar_mul(
                                out=dst, in0=pvt[:, 128 * s:128 * s + 64],
                                scalar1=rcp[:, s:s + 1])
                        else:
                            nc.scalar.activation(
                                out=dst, in_=pvt[:, 128 * s:128 * s + 64],
                                func=Act.Copy, scale=rcp[:, s:s + 1])

            def emit_x1t(j):
                # x1 chunk back to feature-major via PE transpose
                for s in range(2):
                    tp = ps_big.tile([128, 128], bf16, tag="big", name="tp")
                    nc.tensor.transpose(
                        tp[:], x1n[:, s, 128 * j:128 * j + 128],
                        idn[:])
                    if s == 0:
                        nc.vector.tensor_copy(
                            out=x1T[:, j, 128 * s:128 * s + 128], in_=tp[:])
                    else:
                        nc.scalar.copy(
                            out=x1T[:, j, 128 * s:128 * s + 128], in_=tp[:])

            # ---- FFN1: fT = relu(W1 @ x1T (+ b1)), accumulated chunk by
            # chunk as x1T chunks complete. Two u-halves per PSUM bank,
            # one accumulation group per bank. ----
            f_ps = {}

            def emit_ffn1_mms(ec):
                for u in range(4):
                    bk, hf = divmod(u, 2)
                    if ec == 0 and hf == 0:
                        f_ps[bk] = ps_ffn.tile([128, 512], f32, tag="f",
                                               name="fp")
                    nc.tensor.matmul(
                        f_ps[bk][:, 256 * hf:256 * hf + 256],
                        w1_sb[:, ec, 128 * u:128 * u + 128], x1T[:, ec, :],
                        start=(ec == 0 and hf == 0),
                        stop=(ec == 3 and hf == 1))

            def emit_ffn1_fin():
                # split across DVE and ACT so the tail chain shortens
                for u in range(4):
                    bk, hf = divmod(u, 2)
                    src = f_ps[bk][:, 256 * hf:256 * hf + 256]
                    if with_bias:
                        nc.scalar.activation(out=fT[:, u, :], in_=src,
                                             func=Act.Relu,
                                             bias=b1c_sb[:, u:u + 1])
                    elif u % 2 == 0:
                        nc.vector.tensor_scalar_max(out=fT[:, u, :],
                                                    in0=src, scalar1=0.0)
                    else:
                        nc.scalar.activation(out=fT[:, u, :], in_=src,
                                             func=Act.Relu)

            # ---- pipeline, emission order == dependency-arrival order
            # (PE queue is an in-order FIFO: an instruction emitted
            # before its input lands stalls everything behind it) ----
            emit_qproj(0, 0, ps_big)
            emit_kproj(0, 0, ps_big)
            emit_qproj(1, 0, ps_pv)
            emit_kproj(1, 0, ps_pv)
            emit_qproj(0, 1, ps_big)
            emit_kproj(0, 1, ps_big)
            emit_qproj(1, 1, ps_pv)
            emit_kproj(1, 1, ps_pv)
            emit_vproj(0, 0)
            scores = [emit_scores(0)]
            emit_qproj(2, 0, ps_big)
            emit_qproj(2, 1, ps_big)
            emit_kproj(2, 0, ps_big)
            emit_kproj(2, 1, ps_big)
            scores.append(emit_scores(1))
            emit_vproj(0, 1)
            emit_vproj(1, 0)
            emit_vproj(1, 1)
            emit_pv(0, scores[0])
            emit_qproj(3, 0, ps_big)
            emit_qproj(3, 1, ps_big)
            emit_kproj(3, 0, ps_big)
            emit_kproj(3, 1, ps_big)
            emit_x1t(0)
            scores.append(emit_scores(2))
            emit_pv(1, scores[1])
            emit_x1t(1)
            scores.append(emit_scores(3))
            emit_ffn1_mms(0)
            emit_ffn1_mms(1)
            emit_pv(2, scores[2])
            emit_x1t(2)
            emit_ffn1_mms(2)
            emit_pv(3, scores[3])
            emit_x1t(3)
            emit_ffn1_mms(3)
            emit_ffn1_fin()

            # ---- FFN2 + relu (residual added on host) ----
            for qc in range(2):
                gp = ps_big.tile([128, 512], f32, tag="big", name="gp")
                for u in range(4):
                    nc.tensor.matmul(
                        gp[:], fT[:, u, 128 * qc:128 * qc + 128],
                        w2_sb[:, u, :],
                        start=(u == 0), stop=(u == 3))
                if with_bias:
                    nc.vector.tensor_tensor(out=gp[:], in0=gp[:],
                                            in1=b2r_sb[:], op=Alu.add)
                if qc == 0:
                    nc.vector.tensor_scalar_max(out=gout[:, qc, :],
                                                in0=gp[:], scalar1=0.0)
                    nc.sync.dma_start(
                        out_d[128 * qc:128 * qc + 128, :], gout[:, qc, :])
                else:
                    nc.scalar.activation(out=gout[:, qc, :], in_=gp[:],
                                         func=Act.Relu)
                    nc.scalar.dma_start(
                        out_d[128 * qc:128 * qc + 128, :], gout[:, qc, :])
            if DBG:
                nc.sync.dma_start(dbg["dqT"][:], qT[:])
                nc.sync.dma_start(dbg["dkT"][:], kT[:])
                nc.sync.dma_start(dbg["dvsb"][:], v_sb[:])
                nc.sync.dma_start(dbg["dx1n"][:], x1n[:])
                nc.sync.dma_start(dbg["dx1T"][:], x1T[:])
                nc.sync.dma_start(dbg["dfT"][:], fT[:])

    nc.compile()
    return nc


def _get_nc(with_bias):
    key = ("nc", with_bias)
    if key not in _CACHE:
        _CACHE[key] = _build_nc(with_bias)
    return _CACHE[key]


def _to_chunked(w):
    # [512, 512] -> [128, 4*512] with row p = [W[0*128+p,:], W[1*128+p,:],..]
    import ml_dtypes
    bf = ml_dtypes.bfloat16
    return np.ascontiguousarray(
        w.reshape(4, 128, 512).transpose(1, 0, 2).reshape(128, 2048)
        .astype(bf))


def _to_jblocks(w):
    # [512, 512] -> [128, 4j * (4o * 128c)]: j-major blocks, o-major inside
    import ml_dtypes
    bf = ml_dtypes.bfloat16
    # w[o*128+p, j*128+c] -> out[p, j, o, c]
    t = w.reshape(4, 128, 4, 128).transpose(1, 2, 0, 3)
    return np.ascontiguousarray(t.reshape(128, 2048).astype(bf))


def _shard_inputs(x, Wq, Wk, Wv, W1, b1, W2, b2, with_bias):
    import ml_dtypes
    bf = ml_dtypes.bfloat16
    x2 = np.ascontiguousarray(np.asarray(x, dtype=np.float32).reshape(S, E))
    ws = {
        "wq": _to_jblocks(np.asarray(Wq, np.float32).T),
        "wk": _to_jblocks(np.asarray(Wk, np.float32).T),
        "wv": _to_chunked(np.asarray(Wv, np.float32).T),
        "w1": _to_chunked(np.asarray(W1, np.float32).T),
        "w2": _to_chunked(np.asarray(W2, np.float32).T),
        "idn": np.ascontiguousarray(np.eye(128, dtype=np.float32).astype(bf)),
    }
    if with_bias:
        ws["b1c"] = np.ascontiguousarray(
            np.asarray(b1, np.float32).reshape(4, 128).T)
        ws["b2r"] = np.ascontiguousarray(np.broadcast_to(
            np.asarray(b2, np.float32).reshape(1, E), (128, E)))
    in_maps = []
    for c in range(N_CORES):
        r_, half = c >> 1, c & 1
        eidx = np.arange(256 * half - 128, 256 * half + 384)
        valid = (eidx >= 0) & (eidx < S // DIL)
        xe = np.zeros((EXT, E), np.float32)
        xe[valid] = x2[DIL * eidx[valid] + r_]
        in_maps.append({"xet": _to_chunked(xe.T), **ws})
    return in_maps


def _gather_outputs(results, x):
    x2 = np.asarray(x, np.float32).reshape(S, E)
    out = np.zeros((S, E), np.float32)
    for c in range(N_CORES):
        r_, half = c >> 1, c & 1
        i = np.arange(256 * half, 256 * half + SC)
        rows = DIL * i + r_
        out[rows] = np.asarray(results[c]["out"], np.float32) + x2[rows]
    return out.reshape(1, S, E)


def run(inputs, trace=False, tmpdir=None):
    from concourse import bass_utils
    with_bias = bool(np.any(inputs["b1"])) or bool(np.any(inputs["b2"]))
    nc = _get_nc(with_bias)
    in_maps = _shard_inputs(**inputs, with_bias=with_bias)
    res = bass_utils.run_bass_kernel_spmd(
        nc, in_maps, list(range(N_CORES)), trace=trace, tmpdir=tmpdir)
    return _gather_outputs(res.results, inputs["x"]), res


def kernel(x, Wq, Wk, Wv, W1, b1, W2, b2):
    out, _ = run(dict(x=x, Wq=Wq, Wk=Wk, Wv=Wv, W1=W1, b1=b1, W2=W2, b2=b2))
    return out
